# revision 7
# baseline (speedup 1.0000x reference)
"""Trainium2 Bass kernel for nn_LocalizedTokenAggregation.

reference semantics (per full tensors):
  x:   [L=512, B=128, D=512] f32
  tce: [L=512, B=128, C=32] f32
  sim = where(tce > 0, tce, -inf)
  top8 = top_k(sim, 8) over C;  val_min = top8[..., -1]
  sim = where(sim >= val_min, sim, -inf)
  pseudo_label = where(sim > 0, 1, 0)
  s = softmax(sim, axis=0) with all-(-inf) columns -> 0
  merge_val = einsum('lbc,lbd->cbd', s, x)
  returns (merge_val [32,128,512], pseudo_label [512,128,32])

Sharding: data-parallel over B across 8 NeuronCores (16 batches per core).

Per-core layout: l = q*128 + r (q in 0..3, r = partition). Token groups
g = q*16 + b on the SBUF free axis; concepts c innermost.
The equivalent masking used on-chip (tie-free for continuous inputs):
  rl = relu(tce); t = max(8th largest of rl over C, 1e-35)
  mask = (rl >= t)           # == pseudo_label
  n = exp(tce) * mask        # softmax numerator
  denom[b,c] = sum_l n;  merge[c,b,d] = sum_l n * x / max(denom,1e-30)
"""

import sys

sys.path.insert(0, "/opt/trn_rl_repo")

import numpy as np

import concourse.bacc as bacc
import concourse.mybir as mybir
from concourse.tile import TileContext
from concourse.bass_utils import run_bass_kernel_spmd

L, B, D, C = 512, 128, 512, 32
NCORES = 8
BS = B // NCORES  # 16 batches per core
Q = L // 128  # 4 l-chunks
F32 = mybir.dt.float32

# matmul input dtype: float32 (exact, 4 cyc/row) or float32r (1 cyc/row)
MM_DT = mybir.dt.float32

_cache = {}


def build():
    nc = bacc.Bacc("TRN2", target_bir_lowering=False, debug=False)
    x = nc.dram_tensor("x", [L, BS, D], F32, kind="ExternalInput")
    tce = nc.dram_tensor("tce", [L, BS, C], F32, kind="ExternalInput")
    merge = nc.dram_tensor("merge", [C, BS, D], F32, kind="ExternalOutput")
    pseudo = nc.dram_tensor("pseudo", [L, BS, C], F32, kind="ExternalOutput")

    # DRAM views
    # tce/pseudo: (q r) b c -> r (q b) c : partition r, 2KB contiguous runs
    tce_v = tce.ap().rearrange("(q r) b c -> r q b c", r=128)
    pse_v = pseudo.ap().rearrange("(q r) b c -> r q b c", r=128)
    # x: (q r) b d -> q r (b d) : per-q [128, 8192] 4MB contiguous chunks
    x_v = x.ap().rearrange("(q r) b d -> q r (b d)", r=128)
    # merge out per 4-batch group: partition p = bi*32 + c
    mer_v = merge.ap().rearrange("c (g bi) d -> g bi c d", bi=4)

    NG = Q * BS  # 64 token groups of C values per partition

    with TileContext(nc) as tc:
        with (
            tc.tile_pool(name="const", bufs=1) as const,
            tc.tile_pool(name="a", bufs=1) as a,
            tc.tile_pool(name="xs", bufs=2) as xs,
            tc.tile_pool(name="small", bufs=1) as small,
            tc.tile_pool(name="outp", bufs=4) as outp,
            tc.tile_pool(name="ps", bufs=1, space="PSUM") as ps,
        ):
            ones = const.tile([128, 1], F32)
            nc.vector.memset(ones, 1.0)

            # ---- Phase A: per-token concept masking on tce ----
            sim = a.tile([128, NG * C], F32, tag="sim")
            nc.sync.dma_start(out=sim.rearrange("r (q b c) -> r q b c", q=Q, b=BS), in_=tce_v)
            sim3 = sim.rearrange("r (g c) -> r g c", c=C)

            rl = a.tile([128, NG * C], F32, tag="rl")
            nc.scalar.activation(rl, sim, mybir.ActivationFunctionType.Relu)
            rl3 = rl.rearrange("r (g c) -> r g c", c=C)

            ex = a.tile([128, NG * C], F32, tag="ex")
            nc.scalar.activation(ex, sim, mybir.ActivationFunctionType.Exp)

            top8 = a.tile([128, NG, 8], F32, tag="top8")
            for g in range(NG):
                nc.vector.max(out=top8[:, g, :], in_=rl3[:, g, :])

            tval = a.tile([128, NG, 1], F32, tag="tval")
            nc.vector.tensor_scalar_max(tval, top8[:, :, 7:8], 1e-35)

            msk = a.tile([128, NG * C], F32, tag="msk")
            nc.vector.tensor_tensor(
                out=msk.rearrange("r (g c) -> r g c", c=C),
                in0=rl3,
                in1=tval.to_broadcast([128, NG, C]),
                op=mybir.AluOpType.is_ge,
            )

            nt = a.tile([128, NG * C], F32, tag="nt")
            nc.vector.tensor_tensor(
                out=nt, in0=ex, in1=msk, op=mybir.AluOpType.mult
            )

            # pseudo_label is exactly the mask
            nc.gpsimd.dma_start(
                out=pse_v, in_=msk.rearrange("r (q b c) -> r q b c", q=Q, b=BS)
            )

            # ---- Phase B: stream x, matmuls ----
            psg = []
            for g in range(4):
                pt = ps.tile([128, 512], F32, tag=f"psg{g}", name=f"psg{g}")
                psg.append(pt)
            psd = ps.tile([1, 512], F32, tag="psd")

            for q in range(Q):
                xt = xs.tile([128, BS * D], F32, tag="xt")
                nc.sync.dma_start(out=xt, in_=x_v[q])
                # denominator: column sums of n for this l-chunk
                nc.tensor.matmul(
                    psd[0:1, :],
                    lhsT=ones[:, 0:1].bitcast(MM_DT),
                    rhs=nt[:, q * BS * C : (q + 1) * BS * C].bitcast(MM_DT),
                    start=(q == 0),
                    stop=(q == Q - 1),
                )
                for b in range(BS):
                    g, bi = b // 4, b % 4
                    nc.tensor.matmul(
                        psg[g][32 * bi : 32 * (bi + 1), :],
                        lhsT=nt[:, (q * BS + b) * C : (q * BS + b + 1) * C].bitcast(
                            MM_DT
                        ),
                        rhs=xt[:, b * D : (b + 1) * D].bitcast(MM_DT),
                        start=(q == 0),
                        stop=(q == Q - 1),
                        tile_position=(0, 32 * bi),
                    )

            # ---- Tail: 1/denom, scale, store ----
            denom = small.tile([1, 512], F32, tag="denom")
            nc.vector.tensor_scalar_max(denom, psd[0:1, :], 1e-30)
            rec = small.tile([1, 512], F32, tag="rec")
            nc.vector.reciprocal(rec, denom)
            # diagonal transpose: recT[bi*32+c, g] = rec[(g*4+bi)*32+c]
            recT = small.tile([128, 4], F32, tag="recT")
            for g in range(4):
                nc.gpsimd.dma_start(
                    out=recT[:, g : g + 1], in_=rec[0:1, g * 128 : (g + 1) * 128]
                )
            for g in range(4):
                osb = outp.tile([128, 512], F32, tag="osb")
                nc.vector.tensor_scalar_mul(osb, psg[g], recT[:, g : g + 1])
                nc.gpsimd.dma_start(out=mer_v[g], in_=osb)

    nc.compile()
    return nc


def _get_nc():
    if "nc" not in _cache:
        _cache["nc"] = build()
    return _cache["nc"]


def kernel(x: np.ndarray, token_concept_embedding: np.ndarray, **run_kwargs):
    nc = _get_nc()
    x = np.ascontiguousarray(x, dtype=np.float32)
    tce = np.ascontiguousarray(token_concept_embedding, dtype=np.float32)
    in_maps = [
        {
            "x": np.ascontiguousarray(x[:, i * BS : (i + 1) * BS, :]),
            "tce": np.ascontiguousarray(tce[:, i * BS : (i + 1) * BS, :]),
        }
        for i in range(NCORES)
    ]
    res = run_bass_kernel_spmd(nc, in_maps, core_ids=list(range(NCORES)), **run_kwargs)
    merge = np.concatenate([r["merge"] for r in res.results], axis=1)
    pseudo = np.concatenate([r["pseudo"] for r in res.results], axis=1)
    if run_kwargs:
        kernel.last_results = res
    return merge, pseudo
